# revision 3
# baseline (speedup 1.0000x reference)
"""GCN message-passing kernel v3 (nn_Encoder_953482739902) for 8 TRN2 cores.

Design (per core; targets sharded 8 ways, 12500 each, padded 12800):
  - Edges partitioned by target window (128 targets) x source chunk
    (25024 rows, int16-indexable), padded to 128-edge blocks shared
    across cores (max over cores).
  - dma_gather pulls per-edge source rows straight from the HBM fp16
    feature table into [edge_lane, block, feat] SBUF tiles (no table
    residency, no W pre-transform).
  - Per block: one-hot scatter matmul with the one-hot as the
    STATIONARY operand: psw[t, f] += sum_e onehot[e, t] * msg[e, f].
    One-hot = is_equal(iota, tloc) * dinv_src fused on DVE (some built
    on ACT via Relu(dinv - 64*(iota - tloc)^2)).
  - Windows accumulate in PSUM (4 windows per bank). Per window after
    scatter: PSUM->SBUF copy, PE transpose, W matmul (lhsT = scat^T),
    then a single fused Mish activation with per-partition dinv_tgt
    scale, and a direct [t, h] DMA out (no output transpose).

Host does integer/index work + dtype casts only.
"""

import numpy as np

N_NODES = 100000
IN_CH = 128
N_CORES = 8
TPC = 12500
TPAD = 12800
WIN = 128
NW = 98                 # used windows (ceil(12500/128))
CHUNKROWS = 25024
NCHUNK = 4
NPAD = CHUNKROWS * NCHUNK   # 100096
WGRP = 8                # windows per gather group
NGRP = (NW + WGRP - 1) // WGRP   # 13 groups (12x8 + 1x2)
PSG = 4                 # windows per PSUM group
GQUEUES = [int(q) for q in __import__('os').environ.get('GQ', '0,1,2,3').split(',')]
ACT_ONEHOT_EVERY = int(__import__('os').environ.get('ACT_OH', '3'))
BISECT = __import__('os').environ.get('BISECT', '')  # '', 'nogather', 'gatheronly'


def _group_windows(g):
    return list(range(g * WGRP, min((g + 1) * WGRP, NW)))


def _build_schedule(row, col, deg):
    """Partition/sort edges; emit per-core arrays + shared static shapes.

    Returns dict with:
      nb      [NW, NCHUNK] shared block counts
      idx16   [N_CORES, 128, NB*8] int16 wrapped chunk-local gather indices
      tlocb   [N_CORES, 128, NB] fp32 per-block-lane local target (-1 pad)
      ntlocb  [N_CORES, 128, NB] fp32 negated tloc (for ACT path)
      degsb   [N_CORES, 128, NB] fp16 per-block-lane source degree (1 pad)
    """
    core = col // TPC
    tl = col - core * TPC
    w = tl >> 7
    c = row // CHUNKROWS

    key = ((core * NW + w) * NCHUNK + c).astype(np.int64)
    counts = np.bincount(key, minlength=N_CORES * NW * NCHUNK).reshape(
        N_CORES, NW, NCHUNK)
    nb = -(-counts.max(axis=0) // 128)        # [NW, NCHUNK]
    NB = int(nb.sum())

    # sort edges by (core, group, chunk, window) to match gather call order
    g = w // WGRP
    order = np.lexsort((w, c, g, core))
    ro, tlo, wo, co, go, cco = (row[order], tl[order], w[order], c[order],
                                g[order], core[order])

    # cumulative offsets per (core, w, c)
    cum = np.zeros((N_CORES, NW, NCHUNK + 1), np.int64)
    cum[:, :, 1:] = np.cumsum(counts, axis=2)
    wcum = np.zeros((N_CORES, NW + 1), np.int64)
    wcum[:, 1:] = np.cumsum(counts.sum(axis=2), axis=1)
    # edges are sorted (core, g, c, w): build start offset per (core,c,w)
    # via counts in that order
    gidx = np.arange(NW) // WGRP
    # per core: offsets of (g, c, w-within-g) stream
    start = np.zeros((N_CORES, NW, NCHUNK), np.int64)
    for cc in range(N_CORES):
        off = 0
        for gg in range(NGRP):
            for ch in range(NCHUNK):
                for ww in _group_windows(gg):
                    start[cc, ww, ch] = off
                    off += counts[cc, ww, ch]
        assert off == counts[cc].sum()
    core_base = np.zeros(N_CORES + 1, np.int64)
    core_base[1:] = np.cumsum(counts.sum(axis=(1, 2)))

    idx16 = np.zeros((N_CORES, 128, NB * 8), np.int16)
    tlocb = np.full((N_CORES, 128, NB), -1.0, np.float32)
    ntlocb = np.full((N_CORES, 128, NB), 1.0, np.float32)
    degsb = np.ones((N_CORES, 128, NB), np.float16)

    # block column order: for g: for c: for w in g: blocks
    for cc in range(N_CORES):
        iv_all = np.zeros(NB * 128, np.int64)
        tv_all = np.full(NB * 128, -1.0, np.float32)
        dv_all = np.ones(NB * 128, np.float64)
        bcol = 0
        for gg in range(NGRP):
            for ch in range(NCHUNK):
                for ww in _group_windows(gg):
                    nblk = int(nb[ww, ch])
                    if nblk == 0:
                        continue
                    a = core_base[cc] + start[cc, ww, ch]
                    n = int(counts[cc, ww, ch])
                    sl = slice(bcol * 128, bcol * 128 + n)
                    iv_all[sl] = ro[a : a + n] - ch * CHUNKROWS
                    tv_all[sl] = (tlo[a : a + n] - (ww << 7)).astype(np.float32)
                    dv_all[sl] = deg[ro[a : a + n]]
                    bcol += nblk
        assert bcol == NB
        idx16[cc] = np.tile(
            iv_all.astype(np.int16).reshape(-1, 16).T, (8, 1))
        tlocb[cc] = tv_all.reshape(-1, 128).T.astype(np.float32)
        ntlocb[cc] = (-tv_all.reshape(-1, 128).T).astype(np.float32)
        degsb[cc] = dv_all.reshape(-1, 128).T.astype(np.float16)

    return nb, NB, idx16, tlocb, ntlocb, degsb


def _fix_act_table_loads(nc):
    """Pin ACT table to natural_log_exp_and_others; dedup redundant loads."""
    import concourse.mybir as mybir
    from concourse.hw_specs import get_activation_tables

    tables = get_activation_tables(nc.m.arch)
    names = list(tables.keys())
    target = "natural_log_exp_and_others"
    target_id = names.index(target)
    allowed = tables[target]
    for f in nc.m.functions:
        for blk in f.blocks:
            insts = blk.instructions
            for inst in insts:
                if isinstance(inst, mybir.InstActivation):
                    assert inst.func in allowed, inst.func
            kept = []
            seen_load = False
            for inst in insts:
                if isinstance(inst, mybir.InstLoadActFuncSet):
                    si = inst.sync_info
                    has_sync = si is not None and (si.on_wait or si.on_update)
                    if seen_load and not has_sync:
                        continue
                    inst.act_func_set_id = target_id
                    seen_load = True
                kept.append(inst)
            if len(kept) != len(insts):
                insts[:] = kept


def _build_bass(nb, NB):
    import concourse.bacc as bacc
    import concourse.mybir as mybir
    from concourse.tile import TileContext

    AF = mybir.ActivationFunctionType
    OP = mybir.AluOpType
    f16 = mybir.dt.float16
    f32 = mybir.dt.float32

    nc = bacc.Bacc("TRN2", target_bir_lowering=False, debug=False,
                   num_devices=N_CORES, num_swdge_queues=4)
    _orig_compile = nc.compile

    def _compile_with_fix():
        _orig_compile()
        _fix_act_table_loads(nc)

    nc.compile = _compile_with_fix

    x16_d = nc.dram_tensor("x16", [NPAD, IN_CH], f16, kind="ExternalInput")
    idx_d = nc.dram_tensor("idx16", [128, NB * 8], mybir.dt.int16,
                           kind="ExternalInput")
    tloc_d = nc.dram_tensor("tlocb", [128, NB], f32, kind="ExternalInput")
    degs_d = nc.dram_tensor("degsb", [128, NB], f16, kind="ExternalInput")
    degt_d = nc.dram_tensor("degtb", [128, NW], f16, kind="ExternalInput")
    iota_d = nc.dram_tensor("iota128", [128, WIN], f16, kind="ExternalInput")
    w_d = nc.dram_tensor("W16", [IN_CH, IN_CH], f16, kind="ExternalInput")
    id_d = nc.dram_tensor("ident", [128, 128], f32, kind="ExternalInput")
    out_d = nc.dram_tensor("out", [TPAD, IN_CH], f32, kind="ExternalOutput")

    # per-group geometry
    grp_nblk = []     # [g][c] blocks per gather call
    grp_coff = []     # [g][c] column offset of chunk stream in group tile
    grp_total = []    # [g] total blocks in group tile
    blk_map = {}      # w -> list of (g, col_in_group_tile, global_block)
    gb = 0
    icol = 0
    grp_icol = []     # [g][c] idx column start (in idx_d cols, 8 per block)
    for g in range(NGRP):
        ws = _group_windows(g)
        nblk_c = [int(nb[ws, c].sum()) for c in range(NCHUNK)]
        coff_c = np.concatenate([[0], np.cumsum(nblk_c)]).astype(int)
        grp_nblk.append(nblk_c)
        grp_coff.append(coff_c)
        grp_total.append(int(coff_c[-1]))
        grp_icol.append([icol + int(coff_c[c]) * 8 for c in range(NCHUNK)])
        for c in range(NCHUNK):
            pos = int(coff_c[c])
            for w in ws:
                for b in range(int(nb[w, c])):
                    blk_map.setdefault(w, []).append((g, pos, gb))
                    pos += 1
                    gb += 1
        icol += grp_total[g] * 8
    assert gb == NB

    MAXG = max(grp_total)

    with TileContext(nc) as tc:
        with (
            tc.tile_pool(name="const", bufs=1) as cp,
            tc.tile_pool(name="msgp", bufs=2) as mp_,
            tc.tile_pool(name="idxp", bufs=2) as ip_,
            tc.tile_pool(name="ohp", bufs=16) as ohp,
            tc.tile_pool(name="sqp", bufs=4) as sqp,
            tc.tile_pool(name="finp", bufs=2) as fp_,
            tc.tile_pool(name="psW", bufs=3, space="PSUM") as pW,
            tc.tile_pool(name="psT", bufs=2, space="PSUM") as pT,
            tc.tile_pool(name="psO", bufs=2, space="PSUM") as pO,
        ):
            iota = cp.tile([128, WIN], f16)
            nc.sync.dma_start(out=iota[:], in_=iota_d[:])
            wsb = cp.tile([IN_CH, IN_CH], f16)
            nc.sync.dma_start(out=wsb[:], in_=w_d[:])
            ident = cp.tile([128, 128], f32)
            nc.sync.dma_start(out=ident[:], in_=id_d[:])
            tlct = cp.tile([128, NB], f32)
            nc.sync.dma_start(out=tlct[:], in_=tloc_d[:])
            ntlc = cp.tile([128, NB], f32)
            nc.vector.tensor_scalar(out=ntlc[:], in0=tlct[:], scalar1=-1.0,
                                    scalar2=None, op0=OP.mult)
            degs = cp.tile([128, NB], f16)
            nc.sync.dma_start(out=degs[:], in_=degs_d[:])
            degt = cp.tile([128, NW], f16)
            nc.sync.dma_start(out=degt[:], in_=degt_d[:])

            # dinv = exp(-0.5 ln(deg)) for sources (per block lane) and
            # targets (per window column)
            lns = cp.tile([128, NB], f32)
            nc.scalar.activation(lns[:], degs[:], AF.Ln)
            dinvs = cp.tile([128, NB], f32)
            nc.scalar.activation(dinvs[:], lns[:], AF.Exp, scale=-0.5)
            lnt = cp.tile([128, NW], f32)
            nc.scalar.activation(lnt[:], degt[:], AF.Ln)
            dinvt = cp.tile([128, NW], f32)
            nc.scalar.activation(dinvt[:], lnt[:], AF.Exp, scale=-0.5)

            oh_i = 0  # running one-hot counter for engine split
            _qi = [0]  # gather call counter for queue rotation

            for g in range(NGRP):
                ws = _group_windows(g)
                msgs = mp_.tile([128, MAXG, 128], f16, tag="msgs")
                gi0 = grp_icol[g][0]
                gtot = grp_total[g] * 8
                idxt = ip_.tile([128, MAXG * 8], mybir.dt.int16, tag="idxg")
                nc.sync.dma_start(out=idxt[:, :gtot],
                                  in_=idx_d[:, gi0 : gi0 + gtot])
                for c in range(NCHUNK):
                    nblk = grp_nblk[g][c]
                    if nblk == 0:
                        continue
                    a = grp_coff[g][c]
                    ic = grp_icol[g][c] - grp_icol[g][0]
                    if BISECT == 'nogather':
                        nc.vector.memset(msgs[:, a : a + nblk, :], 0.0)
                        continue
                    # SWDGE ring holds 1024 descriptors per queue (hard
                    # ucode limit). single_packet=False rings doorbells
                    # incrementally so SDMA drains while the Q7 generates,
                    # allowing calls up to 64 blocks (8192 descriptors).
                    for s0 in range(0, nblk, 64):
                        sn = min(64, nblk - s0)
                        nc.gpsimd.dma_gather(
                            out_ap=msgs[:, a + s0 : a + s0 + sn, :],
                            in_ap=x16_d[c * CHUNKROWS : (c + 1) * CHUNKROWS, :],
                            idxs_ap=idxt[:, ic + s0 * 8 : ic + (s0 + sn) * 8],
                            num_idxs=sn * 128,
                            num_idxs_reg=sn * 128,
                            elem_size=128,
                            single_packet=False,
                            queue_num=GQUEUES[_qi[0] % len(GQUEUES)])
                        _qi[0] += 1
                if BISECT == 'gatheronly':
                    continue

                # psum groups of PSG windows within this gather group
                for p0 in range(0, len(ws), PSG):
                    wsub = ws[p0 : p0 + PSG]
                    kk = len(wsub)
                    psw = pW.tile([128, kk, 128], f32, tag="psw",
                                  space="PSUM")
                    for k, w in enumerate(wsub):
                        blocks = blk_map.get(w, [])
                        nbw = len(blocks)
                        for j, (gg, col, gbi) in enumerate(blocks):
                            assert gg == g
                            oh = ohp.tile([128, 128], f16, tag="oh")
                            if ACT_ONEHOT_EVERY and (
                                    oh_i % ACT_ONEHOT_EVERY == 0):
                                sq = sqp.tile([128, 128], f32, tag="sq")
                                nc.scalar.activation(
                                    sq[:], iota[:], AF.Square,
                                    bias=ntlc[:, gbi : gbi + 1])
                                nc.scalar.activation(
                                    oh[:], sq[:], AF.Relu,
                                    bias=dinvs[:, gbi : gbi + 1],
                                    scale=-64.0)
                            else:
                                nc.vector.tensor_scalar(
                                    out=oh[:], in0=iota[:],
                                    scalar1=tlct[:, gbi : gbi + 1],
                                    scalar2=dinvs[:, gbi : gbi + 1],
                                    op0=OP.is_equal, op1=OP.mult)
                            oh_i += 1
                            nc.tensor.matmul(
                                out=psw[:, k, :], lhsT=oh[:],
                                rhs=msgs[:, col, :],
                                start=(j == 0), stop=(j == nbw - 1))

                    # transform + finalize
                    sc4 = fp_.tile([128, kk, 128], f32, tag="sc4")
                    nc.scalar.activation(sc4[:], psw[:], AF.Copy)
                    pst = pT.tile([128, kk, 128], f32, tag="pst",
                                  space="PSUM")
                    for k in range(kk):
                        nc.tensor.transpose(pst[:, k, :], sc4[:, k, :],
                                            ident[:])
                    scT = fp_.tile([128, kk, 128], f16, tag="scT")
                    nc.scalar.activation(scT[:], pst[:], AF.Copy)
                    pso = pO.tile([128, kk, 128], f32, tag="pso",
                                  space="PSUM")
                    for k in range(kk):
                        nc.tensor.matmul(out=pso[:, k, :],
                                         lhsT=scT[:, k, :], rhs=wsb[:],
                                         start=True, stop=True)
                    # finalize: mish(dinvt * pso), exact rational form
                    z4 = fp_.tile([128, kk, 128], f32, tag="z4")
                    for k, w in enumerate(wsub):
                        nc.scalar.activation(z4[:, k, :], pso[:, k, :],
                                             AF.Copy,
                                             scale=dinvt[:, w : w + 1])
                    u4 = fp_.tile([128, kk, 128], f32, tag="u4")
                    nc.scalar.activation(u4[:], z4[:], AF.Exp)
                    a4 = fp_.tile([128, kk, 128], f32, tag="a4")
                    nc.vector.scalar_tensor_tensor(
                        out=a4[:], in0=u4[:], scalar=2.0, in1=u4[:],
                        op0=OP.add, op1=OP.mult)
                    d4 = fp_.tile([128, kk, 128], f32, tag="d4")
                    nc.vector.tensor_scalar(out=d4[:], in0=a4[:],
                                            scalar1=2.0, scalar2=None,
                                            op0=OP.add)
                    r4 = fp_.tile([128, kk, 128], f32, tag="r4")
                    nc.vector.reciprocal_approx_fast(out=r4[:], in_=d4[:])
                    m4 = fp_.tile([128, kk, 128], f32, tag="m4")
                    nc.vector.tensor_tensor(out=m4[:], in0=a4[:], in1=r4[:],
                                            op=OP.mult)
                    fin = fp_.tile([128, kk, 128], f32, tag="fin")
                    nc.vector.tensor_tensor(out=fin[:], in0=m4[:],
                                            in1=z4[:], op=OP.mult)
                    for k, w in enumerate(wsub):
                        nc.sync.dma_start(
                            out=out_d[w * WIN : (w + 1) * WIN, :],
                            in_=fin[:, k, :])
    nc.finalize()
    return nc


class _Runner:
    """PJRT runner (axon): jit once, device-resident inputs, reusable."""

    def __init__(self, nc):
        import jax
        import concourse.mybir as mybir
        from jax.sharding import Mesh, PartitionSpec
        from jax.experimental.shard_map import shard_map
        from concourse import bass2jax
        from concourse.bass2jax import _bass_exec_p, install_neuronx_cc_hook

        install_neuronx_cc_hook()
        self.nc = nc
        partition_name = (
            nc.partition_id_tensor.name if nc.partition_id_tensor else None
        )
        in_names, out_names, out_avals, zero_outs = [], [], [], []
        for alloc in nc.m.functions[0].allocations:
            if not isinstance(alloc, mybir.MemoryLocationSet):
                continue
            name = alloc.memorylocations[0].name
            if alloc.kind == "ExternalInput":
                if name != partition_name:
                    in_names.append(name)
            elif alloc.kind == "ExternalOutput":
                shape = tuple(alloc.tensor_shape)
                dtype = mybir.dt.np(alloc.dtype)
                out_names.append(name)
                out_avals.append(jax.core.ShapedArray(shape, dtype))
                zero_outs.append(np.zeros(shape, dtype))
        self.in_names, self.out_names = in_names, out_names
        all_in = list(in_names) + list(out_names)
        if partition_name is not None:
            all_in.append(partition_name)

        def _body(*args):
            operands = list(args)
            if partition_name is not None:
                operands.append(bass2jax.partition_id_tensor())
            return tuple(_bass_exec_p.bind(
                *operands,
                out_avals=tuple(out_avals),
                in_names=tuple(all_in),
                out_names=tuple(out_names),
                lowering_input_output_aliases=(),
                sim_require_finite=True,
                sim_require_nnan=True,
                nc=nc,
            ))

        import os
        plat = os.environ.get("BASS_MESH_PLATFORM")
        devices = (jax.devices(plat) if plat else jax.devices())[:N_CORES]
        mesh = Mesh(np.asarray(devices), ("core",))
        n_in = len(in_names) + len(out_names)
        self.fn = jax.jit(
            shard_map(_body, mesh=mesh,
                      in_specs=(PartitionSpec("core"),) * n_in,
                      out_specs=(PartitionSpec("core"),) * len(out_names),
                      check_rep=False),
            keep_unused=True)
        self.zero_outs = zero_outs
        self.jax = jax

    def stage(self, in_maps):
        args = []
        for name in self.in_names:
            args.append(np.concatenate(
                [np.asarray(m[name]) for m in in_maps], axis=0))
        for z in self.zero_outs:
            args.append(np.concatenate([z] * N_CORES, axis=0))
        self._dev_args = [self.jax.device_put(a) for a in args]
        for a in self._dev_args:
            a.block_until_ready()

    def run(self):
        outs = self.fn(*self._dev_args)
        for o in outs:
            o.block_until_ready()
        return outs

    def results(self, outs):
        per_core = [dict() for _ in range(N_CORES)]
        for i, name in enumerate(self.out_names):
            arr = np.asarray(outs[i])
            for c, piece in enumerate(np.split(arr, N_CORES, axis=0)):
                per_core[c][name] = piece
        return per_core


_CACHE = {}


def _prepare(x, edge_index, W, b):
    x = np.asarray(x, dtype=np.float32)
    edge_index = np.asarray(edge_index)
    W_ = np.asarray(W, dtype=np.float32)

    row = edge_index[0].astype(np.int64)
    col = edge_index[1].astype(np.int64)
    loops = np.arange(N_NODES, dtype=np.int64)
    row_all = np.concatenate([row, loops])
    col_all = np.concatenate([col, loops])

    deg = np.bincount(col_all, minlength=N_NODES).astype(np.int64)

    nb, NB, idx16, tlocb, ntlocb, degsb = _build_schedule(
        row_all, col_all, deg)

    x16 = np.zeros((NPAD, IN_CH), np.float16)
    x16[:N_NODES] = x.astype(np.float16)

    degt_b = np.ones((N_CORES, 128, NW), np.float16)
    for c in range(N_CORES):
        d = np.ones(NW * 128, np.float16)
        d[:TPC] = deg[c * TPC : (c + 1) * TPC].astype(np.float16)
        degt_b[c] = d.reshape(NW, 128).T

    iota = np.broadcast_to(np.arange(WIN, dtype=np.float16), (128, WIN)).copy()
    ident = np.eye(128, dtype=np.float32)
    w16 = W_.astype(np.float16)

    in_maps = []
    for c in range(N_CORES):
        in_maps.append({
            "x16": x16,
            "idx16": np.ascontiguousarray(idx16[c]),
            "tlocb": np.ascontiguousarray(tlocb[c]),
            "degsb": np.ascontiguousarray(degsb[c]),
            "degtb": np.ascontiguousarray(degt_b[c]),
            "iota128": iota,
            "W16": w16,
            "ident": ident,
        })

    key = (NB, tuple(nb.ravel()))
    if key not in _CACHE:
        nc = _build_bass(nb, NB)
        runner = _Runner(nc)
        _CACHE.clear()
        _CACHE[key] = runner
    return _CACHE[key], in_maps


def kernel(x, edge_index, W, b):
    runner, in_maps = _prepare(x, edge_index, W, b)
    runner.stage(in_maps)
    outs = runner.run()
    res = runner.results(outs)
    return np.concatenate(
        [res[c]["out"][:TPC] for c in range(N_CORES)], axis=0)


# revision 4
# speedup vs baseline: 1.1765x; 1.1765x over previous
"""GCN message-passing kernel v3 (nn_Encoder_953482739902) for 8 TRN2 cores.

Design (per core; targets sharded 8 ways, 12500 each, padded 12800):
  - Edges partitioned by target window (128 targets) x source chunk
    (25024 rows, int16-indexable), padded to 128-edge blocks shared
    across cores (max over cores).
  - dma_gather pulls per-edge source rows straight from the HBM fp16
    feature table into [edge_lane, block, feat] SBUF tiles (no table
    residency, no W pre-transform).
  - Per block: one-hot scatter matmul with the one-hot as the
    STATIONARY operand: psw[t, f] += sum_e onehot[e, t] * msg[e, f].
    One-hot = is_equal(iota, tloc) * dinv_src fused on DVE (some built
    on ACT via Relu(dinv - 64*(iota - tloc)^2)).
  - Windows accumulate in PSUM (4 windows per bank). Per window after
    scatter: PSUM->SBUF copy, PE transpose, W matmul (lhsT = scat^T),
    then a single fused Mish activation with per-partition dinv_tgt
    scale, and a direct [t, h] DMA out (no output transpose).

Host does integer/index work + dtype casts only.
"""

import numpy as np

N_NODES = 100000
IN_CH = 128
N_CORES = 8
TPC = 12500
TPAD = 12800
WIN = 128
NW = 98                 # used windows (ceil(12500/128))
CHUNKROWS = 25024
NCHUNK = 4
NPAD = CHUNKROWS * NCHUNK   # 100096
WGRP = 8                # windows per gather group
NGRP = (NW + WGRP - 1) // WGRP   # 13 groups (12x8 + 1x2)
PSG = 4                 # windows per PSUM group
GQUEUES = [int(q) for q in __import__('os').environ.get('GQ', '0,1,2,3').split(',')]
ACT_ONEHOT_EVERY = int(__import__('os').environ.get('ACT_OH', '4'))
BISECT = __import__('os').environ.get('BISECT', '')  # '', 'nogather', 'gatheronly'


def _group_windows(g):
    return list(range(g * WGRP, min((g + 1) * WGRP, NW)))


def _build_schedule(row, col, deg):
    """Partition/sort edges; emit per-core arrays + shared static shapes.

    Returns dict with:
      nb      [NW, NCHUNK] shared block counts
      idx16   [N_CORES, 128, NB*8] int16 wrapped chunk-local gather indices
      tlocb   [N_CORES, 128, NB] fp32 per-block-lane local target (-1 pad)
      ntlocb  [N_CORES, 128, NB] fp32 negated tloc (for ACT path)
      degsb   [N_CORES, 128, NB] fp16 per-block-lane source degree (1 pad)
    """
    core = col // TPC
    tl = col - core * TPC
    w = tl >> 7
    c = row // CHUNKROWS

    key = ((core * NW + w) * NCHUNK + c).astype(np.int64)
    counts = np.bincount(key, minlength=N_CORES * NW * NCHUNK).reshape(
        N_CORES, NW, NCHUNK)
    nb = -(-counts.max(axis=0) // 128)        # [NW, NCHUNK]
    NB = int(nb.sum())

    # sort edges by (core, group, chunk, window) to match gather call order
    g = w // WGRP
    order = np.lexsort((w, c, g, core))
    ro, tlo, wo, co, go, cco = (row[order], tl[order], w[order], c[order],
                                g[order], core[order])

    # cumulative offsets per (core, w, c)
    cum = np.zeros((N_CORES, NW, NCHUNK + 1), np.int64)
    cum[:, :, 1:] = np.cumsum(counts, axis=2)
    wcum = np.zeros((N_CORES, NW + 1), np.int64)
    wcum[:, 1:] = np.cumsum(counts.sum(axis=2), axis=1)
    # edges are sorted (core, g, c, w): build start offset per (core,c,w)
    # via counts in that order
    gidx = np.arange(NW) // WGRP
    # per core: offsets of (g, c, w-within-g) stream
    start = np.zeros((N_CORES, NW, NCHUNK), np.int64)
    for cc in range(N_CORES):
        off = 0
        for gg in range(NGRP):
            for ch in range(NCHUNK):
                for ww in _group_windows(gg):
                    start[cc, ww, ch] = off
                    off += counts[cc, ww, ch]
        assert off == counts[cc].sum()
    core_base = np.zeros(N_CORES + 1, np.int64)
    core_base[1:] = np.cumsum(counts.sum(axis=(1, 2)))

    idx16 = np.zeros((N_CORES, 128, NB * 8), np.int16)
    tlocb = np.full((N_CORES, 128, NB), -1.0, np.float32)
    ntlocb = np.full((N_CORES, 128, NB), 1.0, np.float32)
    degsb = np.ones((N_CORES, 128, NB), np.float16)

    # block column order: for g: for c: for w in g: blocks
    for cc in range(N_CORES):
        iv_all = np.zeros(NB * 128, np.int64)
        tv_all = np.full(NB * 128, -1.0, np.float32)
        dv_all = np.ones(NB * 128, np.float64)
        bcol = 0
        for gg in range(NGRP):
            for ch in range(NCHUNK):
                for ww in _group_windows(gg):
                    nblk = int(nb[ww, ch])
                    if nblk == 0:
                        continue
                    a = core_base[cc] + start[cc, ww, ch]
                    n = int(counts[cc, ww, ch])
                    sl = slice(bcol * 128, bcol * 128 + n)
                    iv_all[sl] = ro[a : a + n] - ch * CHUNKROWS
                    tv_all[sl] = (tlo[a : a + n] - (ww << 7)).astype(np.float32)
                    dv_all[sl] = deg[ro[a : a + n]]
                    bcol += nblk
        assert bcol == NB
        idx16[cc] = np.tile(
            iv_all.astype(np.int16).reshape(-1, 16).T, (8, 1))
        tlocb[cc] = tv_all.reshape(-1, 128).T.astype(np.float32)
        ntlocb[cc] = (-tv_all.reshape(-1, 128).T).astype(np.float32)
        degsb[cc] = dv_all.reshape(-1, 128).T.astype(np.float16)

    return nb, NB, idx16, tlocb, ntlocb, degsb


def _fix_act_table_loads(nc):
    """Pin ACT table to natural_log_exp_and_others; dedup redundant loads."""
    import concourse.mybir as mybir
    from concourse.hw_specs import get_activation_tables

    tables = get_activation_tables(nc.m.arch)
    names = list(tables.keys())
    target = "natural_log_exp_and_others"
    target_id = names.index(target)
    allowed = tables[target]
    for f in nc.m.functions:
        for blk in f.blocks:
            insts = blk.instructions
            for inst in insts:
                if isinstance(inst, mybir.InstActivation):
                    assert inst.func in allowed, inst.func
            kept = []
            seen_load = False
            for inst in insts:
                if isinstance(inst, mybir.InstLoadActFuncSet):
                    si = inst.sync_info
                    has_sync = si is not None and (si.on_wait or si.on_update)
                    if seen_load and not has_sync:
                        continue
                    inst.act_func_set_id = target_id
                    seen_load = True
                kept.append(inst)
            if len(kept) != len(insts):
                insts[:] = kept


def _build_bass(nb, NB):
    import concourse.bacc as bacc
    import concourse.mybir as mybir
    from concourse.tile import TileContext

    AF = mybir.ActivationFunctionType
    OP = mybir.AluOpType
    f16 = mybir.dt.float16
    f32 = mybir.dt.float32

    nc = bacc.Bacc("TRN2", target_bir_lowering=False, debug=False,
                   num_devices=N_CORES, num_swdge_queues=4)
    _orig_compile = nc.compile

    def _compile_with_fix():
        _orig_compile()
        _fix_act_table_loads(nc)

    nc.compile = _compile_with_fix

    x16_d = nc.dram_tensor("x16", [NPAD, IN_CH], f16, kind="ExternalInput")
    idx_d = nc.dram_tensor("idx16", [128, NB * 8], mybir.dt.int16,
                           kind="ExternalInput")
    tloc_d = nc.dram_tensor("tlocb", [128, NB], f32, kind="ExternalInput")
    degs_d = nc.dram_tensor("degsb", [128, NB], f16, kind="ExternalInput")
    degt_d = nc.dram_tensor("degtb", [128, NW], f16, kind="ExternalInput")
    iota_d = nc.dram_tensor("iota128", [128, WIN], f16, kind="ExternalInput")
    w_d = nc.dram_tensor("W16", [IN_CH, IN_CH], f16, kind="ExternalInput")
    id_d = nc.dram_tensor("ident", [128, 128], f32, kind="ExternalInput")
    out_d = nc.dram_tensor("out", [TPAD, IN_CH], f32, kind="ExternalOutput")

    # per-group geometry
    grp_nblk = []     # [g][c] blocks per gather call
    grp_coff = []     # [g][c] column offset of chunk stream in group tile
    grp_total = []    # [g] total blocks in group tile
    blk_map = {}      # w -> list of (g, col_in_group_tile, global_block)
    gb = 0
    icol = 0
    grp_icol = []     # [g][c] idx column start (in idx_d cols, 8 per block)
    for g in range(NGRP):
        ws = _group_windows(g)
        nblk_c = [int(nb[ws, c].sum()) for c in range(NCHUNK)]
        coff_c = np.concatenate([[0], np.cumsum(nblk_c)]).astype(int)
        grp_nblk.append(nblk_c)
        grp_coff.append(coff_c)
        grp_total.append(int(coff_c[-1]))
        grp_icol.append([icol + int(coff_c[c]) * 8 for c in range(NCHUNK)])
        for c in range(NCHUNK):
            pos = int(coff_c[c])
            for w in ws:
                for b in range(int(nb[w, c])):
                    blk_map.setdefault(w, []).append((g, pos, gb))
                    pos += 1
                    gb += 1
        icol += grp_total[g] * 8
    assert gb == NB

    MAXG = max(grp_total)

    with TileContext(nc) as tc:
        with (
            tc.tile_pool(name="const", bufs=1) as cp,
            tc.tile_pool(name="msgp", bufs=2) as mp_,
            tc.tile_pool(name="idxp", bufs=2) as ip_,
            tc.tile_pool(name="ohp", bufs=8) as ohp,
            tc.tile_pool(name="sqp", bufs=4) as sqp,
            tc.tile_pool(name="finp", bufs=2) as fp_,
            tc.tile_pool(name="psW", bufs=2, space="PSUM") as pW,
            tc.tile_pool(name="psT", bufs=2, space="PSUM") as pT,
            tc.tile_pool(name="psO", bufs=2, space="PSUM") as pO,
        ):
            iota = cp.tile([128, WIN], f16)
            nc.sync.dma_start(out=iota[:], in_=iota_d[:])
            wsb = cp.tile([IN_CH, IN_CH], f16)
            nc.sync.dma_start(out=wsb[:], in_=w_d[:])
            ident = cp.tile([128, 128], f32)
            nc.sync.dma_start(out=ident[:], in_=id_d[:])
            tlct = cp.tile([128, NB], f32)
            nc.sync.dma_start(out=tlct[:], in_=tloc_d[:])
            ntlc = cp.tile([128, NB], f32)
            nc.vector.tensor_scalar(out=ntlc[:], in0=tlct[:], scalar1=-1.0,
                                    scalar2=None, op0=OP.mult)
            degs = cp.tile([128, NB], f16)
            nc.sync.dma_start(out=degs[:], in_=degs_d[:])
            degt = cp.tile([128, NW], f16)
            nc.sync.dma_start(out=degt[:], in_=degt_d[:])

            # dinv = exp(-0.5 ln(deg)) for sources (per block lane) and
            # targets (per window column)
            lns = cp.tile([128, NB], f32)
            nc.scalar.activation(lns[:], degs[:], AF.Ln)
            dinvs = cp.tile([128, NB], f32)
            nc.scalar.activation(dinvs[:], lns[:], AF.Exp, scale=-0.5)
            lnt = cp.tile([128, NW], f32)
            nc.scalar.activation(lnt[:], degt[:], AF.Ln)
            dinvt = cp.tile([128, NW], f32)
            nc.scalar.activation(dinvt[:], lnt[:], AF.Exp, scale=-0.5)

            oh_i = 0  # running one-hot counter for engine split
            _qi = [0]  # gather call counter for queue rotation

            for g in range(NGRP):
                ws = _group_windows(g)
                msgs = mp_.tile([128, MAXG, 128], f16, tag="msgs")
                gi0 = grp_icol[g][0]
                gtot = grp_total[g] * 8
                idxt = ip_.tile([128, MAXG * 8], mybir.dt.int16, tag="idxg")
                nc.sync.dma_start(out=idxt[:, :gtot],
                                  in_=idx_d[:, gi0 : gi0 + gtot])
                for c in range(NCHUNK):
                    nblk = grp_nblk[g][c]
                    if nblk == 0:
                        continue
                    a = grp_coff[g][c]
                    ic = grp_icol[g][c] - grp_icol[g][0]
                    if BISECT == 'nogather':
                        nc.vector.memset(msgs[:, a : a + nblk, :], 0.0)
                        continue
                    # SWDGE ring holds 1024 descriptors per queue (hard
                    # ucode limit). single_packet=False rings doorbells
                    # incrementally so SDMA drains while the Q7 generates,
                    # allowing calls up to 64 blocks (8192 descriptors).
                    for s0 in range(0, nblk, 64):
                        sn = min(64, nblk - s0)
                        nc.gpsimd.dma_gather(
                            out_ap=msgs[:, a + s0 : a + s0 + sn, :],
                            in_ap=x16_d[c * CHUNKROWS : (c + 1) * CHUNKROWS, :],
                            idxs_ap=idxt[:, ic + s0 * 8 : ic + (s0 + sn) * 8],
                            num_idxs=sn * 128,
                            num_idxs_reg=sn * 128,
                            elem_size=128,
                            single_packet=False,
                            queue_num=GQUEUES[_qi[0] % len(GQUEUES)])
                        _qi[0] += 1
                if BISECT == 'gatheronly':
                    continue

                # psum groups of PSG windows within this gather group
                for p0 in range(0, len(ws), PSG):
                    wsub = ws[p0 : p0 + PSG]
                    kk = len(wsub)
                    psw = pW.tile([128, kk, 128], f32, tag="psw",
                                  space="PSUM")
                    for k, w in enumerate(wsub):
                        blocks = blk_map.get(w, [])
                        nbw = len(blocks)
                        for j, (gg, col, gbi) in enumerate(blocks):
                            assert gg == g
                            oh = ohp.tile([128, 128], f16, tag="oh")
                            if ACT_ONEHOT_EVERY and (
                                    oh_i % ACT_ONEHOT_EVERY == 0):
                                sq = sqp.tile([128, 128], f32, tag="sq")
                                nc.scalar.activation(
                                    sq[:], iota[:], AF.Square,
                                    bias=ntlc[:, gbi : gbi + 1])
                                nc.scalar.activation(
                                    oh[:], sq[:], AF.Relu,
                                    bias=dinvs[:, gbi : gbi + 1],
                                    scale=-64.0)
                            else:
                                nc.vector.tensor_scalar(
                                    out=oh[:], in0=iota[:],
                                    scalar1=tlct[:, gbi : gbi + 1],
                                    scalar2=dinvs[:, gbi : gbi + 1],
                                    op0=OP.is_equal, op1=OP.mult)
                            oh_i += 1
                            nc.tensor.matmul(
                                out=psw[:, k, :], lhsT=oh[:],
                                rhs=msgs[:, col, :],
                                start=(j == 0), stop=(j == nbw - 1))

                    # transform + finalize
                    sc4 = fp_.tile([128, kk, 128], f32, tag="sc4")
                    nc.scalar.activation(sc4[:], psw[:], AF.Copy)
                    pst = pT.tile([128, kk, 128], f32, tag="pst",
                                  space="PSUM")
                    for k in range(kk):
                        nc.tensor.transpose(pst[:, k, :], sc4[:, k, :],
                                            ident[:])
                    scT = fp_.tile([128, kk, 128], f16, tag="scT")
                    nc.scalar.activation(scT[:], pst[:], AF.Copy)
                    pso = pO.tile([128, kk, 128], f32, tag="pso",
                                  space="PSUM")
                    for k in range(kk):
                        nc.tensor.matmul(out=pso[:, k, :],
                                         lhsT=scT[:, k, :], rhs=wsb[:],
                                         start=True, stop=True)
                    # finalize: mish(dinvt * pso), exact rational form
                    z4 = fp_.tile([128, kk, 128], f32, tag="z4")
                    for k, w in enumerate(wsub):
                        nc.scalar.activation(z4[:, k, :], pso[:, k, :],
                                             AF.Copy,
                                             scale=dinvt[:, w : w + 1])
                    u4 = fp_.tile([128, kk, 128], f32, tag="u4")
                    nc.scalar.activation(u4[:], z4[:], AF.Exp)
                    a4 = fp_.tile([128, kk, 128], f32, tag="a4")
                    nc.vector.scalar_tensor_tensor(
                        out=a4[:], in0=u4[:], scalar=2.0, in1=u4[:],
                        op0=OP.add, op1=OP.mult)
                    d4 = fp_.tile([128, kk, 128], f32, tag="d4")
                    nc.vector.tensor_scalar(out=d4[:], in0=a4[:],
                                            scalar1=2.0, scalar2=None,
                                            op0=OP.add)
                    r4 = fp_.tile([128, kk, 128], f32, tag="r4")
                    nc.vector.reciprocal_approx_fast(out=r4[:], in_=d4[:])
                    m4 = fp_.tile([128, kk, 128], f32, tag="m4")
                    nc.vector.tensor_tensor(out=m4[:], in0=a4[:], in1=r4[:],
                                            op=OP.mult)
                    fin = fp_.tile([128, kk, 128], f32, tag="fin")
                    nc.vector.tensor_tensor(out=fin[:], in0=m4[:],
                                            in1=z4[:], op=OP.mult)
                    for k, w in enumerate(wsub):
                        nc.sync.dma_start(
                            out=out_d[w * WIN : (w + 1) * WIN, :],
                            in_=fin[:, k, :])
    nc.finalize()
    return nc


class _Runner:
    """PJRT runner (axon): jit once, device-resident inputs, reusable."""

    def __init__(self, nc):
        import jax
        import concourse.mybir as mybir
        from jax.sharding import Mesh, PartitionSpec
        from jax.experimental.shard_map import shard_map
        from concourse import bass2jax
        from concourse.bass2jax import _bass_exec_p, install_neuronx_cc_hook

        install_neuronx_cc_hook()
        self.nc = nc
        partition_name = (
            nc.partition_id_tensor.name if nc.partition_id_tensor else None
        )
        in_names, out_names, out_avals, zero_outs = [], [], [], []
        for alloc in nc.m.functions[0].allocations:
            if not isinstance(alloc, mybir.MemoryLocationSet):
                continue
            name = alloc.memorylocations[0].name
            if alloc.kind == "ExternalInput":
                if name != partition_name:
                    in_names.append(name)
            elif alloc.kind == "ExternalOutput":
                shape = tuple(alloc.tensor_shape)
                dtype = mybir.dt.np(alloc.dtype)
                out_names.append(name)
                out_avals.append(jax.core.ShapedArray(shape, dtype))
                zero_outs.append(np.zeros(shape, dtype))
        self.in_names, self.out_names = in_names, out_names
        all_in = list(in_names) + list(out_names)
        if partition_name is not None:
            all_in.append(partition_name)

        def _body(*args):
            operands = list(args)
            if partition_name is not None:
                operands.append(bass2jax.partition_id_tensor())
            return tuple(_bass_exec_p.bind(
                *operands,
                out_avals=tuple(out_avals),
                in_names=tuple(all_in),
                out_names=tuple(out_names),
                lowering_input_output_aliases=(),
                sim_require_finite=True,
                sim_require_nnan=True,
                nc=nc,
            ))

        import os
        plat = os.environ.get("BASS_MESH_PLATFORM")
        devices = (jax.devices(plat) if plat else jax.devices())[:N_CORES]
        mesh = Mesh(np.asarray(devices), ("core",))
        n_in = len(in_names) + len(out_names)
        self.fn = jax.jit(
            shard_map(_body, mesh=mesh,
                      in_specs=(PartitionSpec("core"),) * n_in,
                      out_specs=(PartitionSpec("core"),) * len(out_names),
                      check_rep=False),
            keep_unused=True)
        self.zero_outs = zero_outs
        self.jax = jax

    def stage(self, in_maps):
        args = []
        for name in self.in_names:
            args.append(np.concatenate(
                [np.asarray(m[name]) for m in in_maps], axis=0))
        for z in self.zero_outs:
            args.append(np.concatenate([z] * N_CORES, axis=0))
        self._dev_args = [self.jax.device_put(a) for a in args]
        for a in self._dev_args:
            a.block_until_ready()

    def run(self):
        outs = self.fn(*self._dev_args)
        for o in outs:
            o.block_until_ready()
        return outs

    def results(self, outs):
        per_core = [dict() for _ in range(N_CORES)]
        for i, name in enumerate(self.out_names):
            arr = np.asarray(outs[i])
            for c, piece in enumerate(np.split(arr, N_CORES, axis=0)):
                per_core[c][name] = piece
        return per_core


_CACHE = {}


def _prepare(x, edge_index, W, b):
    x = np.asarray(x, dtype=np.float32)
    edge_index = np.asarray(edge_index)
    W_ = np.asarray(W, dtype=np.float32)

    row = edge_index[0].astype(np.int64)
    col = edge_index[1].astype(np.int64)
    loops = np.arange(N_NODES, dtype=np.int64)
    row_all = np.concatenate([row, loops])
    col_all = np.concatenate([col, loops])

    deg = np.bincount(col_all, minlength=N_NODES).astype(np.int64)

    nb, NB, idx16, tlocb, ntlocb, degsb = _build_schedule(
        row_all, col_all, deg)

    x16 = np.zeros((NPAD, IN_CH), np.float16)
    x16[:N_NODES] = x.astype(np.float16)

    degt_b = np.ones((N_CORES, 128, NW), np.float16)
    for c in range(N_CORES):
        d = np.ones(NW * 128, np.float16)
        d[:TPC] = deg[c * TPC : (c + 1) * TPC].astype(np.float16)
        degt_b[c] = d.reshape(NW, 128).T

    iota = np.broadcast_to(np.arange(WIN, dtype=np.float16), (128, WIN)).copy()
    ident = np.eye(128, dtype=np.float32)
    w16 = W_.astype(np.float16)

    in_maps = []
    for c in range(N_CORES):
        in_maps.append({
            "x16": x16,
            "idx16": np.ascontiguousarray(idx16[c]),
            "tlocb": np.ascontiguousarray(tlocb[c]),
            "degsb": np.ascontiguousarray(degsb[c]),
            "degtb": np.ascontiguousarray(degt_b[c]),
            "iota128": iota,
            "W16": w16,
            "ident": ident,
        })

    key = (NB, tuple(nb.ravel()))
    if key not in _CACHE:
        nc = _build_bass(nb, NB)
        runner = _Runner(nc)
        _CACHE.clear()
        _CACHE[key] = runner
    return _CACHE[key], in_maps


def kernel(x, edge_index, W, b):
    runner, in_maps = _prepare(x, edge_index, W, b)
    runner.stage(in_maps)
    outs = runner.run()
    res = runner.results(outs)
    return np.concatenate(
        [res[c]["out"][:TPC] for c in range(N_CORES)], axis=0)
